# revision 11
# baseline (speedup 1.0000x reference)
"""C3DLoss kernel for Trainium2 — 8-core batch-parallel, raw-Bass implementation.

Per core = one batch frame b (pairing partner tb = b^1):
    partial = sum over terms t in {same, cross}, shifts d in [-2,2]^2, pixels p:
        exp(-50*(|feat_r(p) - feat_q(p+d)|^2))
    where feat = (xyz, rgb, m) with m = +20*(1-mask_ref) on the ref side and
    -20*(1-mask_query) on the query side; the cross term of the m channel
    makes the squared distance >= 400 whenever either mask is off, so
    exp(-50*d2) -> 0 exactly like the reference's mask products.
    loss = -(sum of partials) / max(sum(depth_gt_mask), 1).

Device mapping (v2):
  - Host pre-blocks every plane into G=16 W-blocks of width 76 with +-2 halo
    in both dims, fp16. All 7 channels x 16 blocks = 112 partitions in ONE
    tile, so each shift needs exactly one DVE subtract (fp16, 2x mode) and
    one square.
  - Squares are statically split across DVE (tensor_mul), ACT (Square
    activation), and Pool/GPSIMD (tensor_tensor mult) to balance engines.
  - PE reduces channels per slot with [112,32] selector weights that write
    each block's sum to 2 duplicated output partitions (tile_position must
    be 32-aligned); 4 slots pack a 128-partition PSUM bank; one matmul per
    (slot, 6-row chunk), start=stop=True.
  - ACT computes exp(-50*d2) in place on PSUM with multi-bank access
    patterns and accumulates into acc columns; host divides the total by 2
    (duplication) and by n_gt.
"""

import sys

for _p in ("/opt/trn_rl_repo", "/opt/pypackages"):
    if _p not in sys.path:
        sys.path.insert(0, _p)

from contextlib import ExitStack

import numpy as np

import concourse.bass as bass
import concourse.mybir as mybir
from concourse.ap import AP
from concourse.alu_op_type import AluOpType

F16 = mybir.dt.float16
F32 = mybir.dt.float32

R = 2
G = 16            # W-blocks
CH = 7            # x, y, z, r, g, b, mask
P = CH * G        # 112 partitions per feature tile
SBATCH = 4        # slots per 128-partition PSUM bank
NSQ = 16          # rotating diff/square tiles
M_C = 20.0        # mask channel magnitude (20+20)^2 = 1600 >> exp cutoff
EXP_SCALE = -50.0

# per-slab square-engine assignment counts (DVE, ACT, Pool)
N_SQ_DVE = 12
N_SQ_ACT = 21
N_SQ_POOL = 17


class Cfg:
    def __init__(self, H=352, W=1216, HS=32):
        assert W % G == 0 and H % HS == 0
        self.H, self.W, self.HS = H, W, HS
        self.WB = W // G                       # 76
        self.WBH = self.WB + 2 * R             # 80
        self.Hp = H + 2 * R                    # 356
        self.NSLAB = H // HS                   # 11
        self.NQ = G * self.Hp * self.WBH       # haloed plane elems
        self.QF = (HS + 2 * R) * self.WBH      # query tile free size 2880
        self.SF = HS * self.WBH                # slab tile free size 2560
        # 6-row chunks: nr*WB <= 512 PSUM cols
        cr = 512 // self.WB                    # 6
        self.chunks = []
        o = 0
        while o < HS:
            self.chunks.append((o, min(cr, HS - o)))
            o += cr
        self.NC = len(self.chunks)             # 6
        self.slots = [(t, dy, dx) for t in (0, 1)
                      for dy in range(-R, R + 1) for dx in range(-R, R + 1)]
        self.NS = len(self.slots)              # 50
        self.batches = [self.slots[i:i + SBATCH]
                        for i in range(0, self.NS, SBATCH)]
        self.NB = len(self.batches)            # 13
        self.GB = self.NSLAB * self.NB         # 143 global batches
        self.NU = self.GB * self.NC            # 858 global units (bank uses)

        # square-engine assignment per slab slot (largest-remainder interleave)
        want = {'D': N_SQ_DVE, 'A': N_SQ_ACT, 'P': N_SQ_POOL}
        assert sum(want.values()) == self.NS
        emitted = {k: 0 for k in want}
        self.assign = []
        for j in range(self.NS):
            k = max(want, key=lambda e: want[e] * (j + 1) / self.NS - emitted[e])
            emitted[k] += 1
            self.assign.append(k)
        # cumulative per-engine counts over global slot index
        self.cum = {'D': [], 'A': [], 'P': []}
        tot = {'D': 0, 'A': 0, 'P': 0}
        self.NSLOT = self.NSLAB * self.NS
        for gj in range(self.NSLOT):
            e = self.assign[gj % self.NS]
            tot[e] += 1
            for k in self.cum:
                self.cum[k].append(tot[k])

        # batch of a global slot
        def gbatch(gj):
            s, j = divmod(gj, self.NS)
            return s * self.NB + j // SBATCH
        self.gbatch = gbatch

        # exp ops: per global batch, group 456-col chunks into
        # bank-contiguous runs; the short chunk is always separate.
        self.exp_ops = []                      # (gb, c0, c1, bank0, cols, pb)
        self.expop_of_unit = [None] * self.NU
        for gb in range(self.GB):
            nslots = len(self.batches[gb % self.NB])
            pb = 32 * nslots
            runs = []
            for c in range(self.NC):
                ro, nr = self.chunks[c]
                k = (gb * self.NC + c) % 8
                cols = nr * self.WB
                if runs and cols == 456 and runs[-1][3] == 456 \
                        and k == runs[-1][2] + (c - runs[-1][0]):
                    runs[-1][1] = c          # extend bank-contiguous run
                else:
                    runs.append([c, c, k, cols])
            for (c0, c1, k0, cols) in runs:
                e = len(self.exp_ops)
                self.exp_ops.append((gb, c0, c1, k0, cols, pb))
                for c in range(c0, c1 + 1):
                    self.expop_of_unit[gb * self.NC + c] = e
        self.NEXP = len(self.exp_ops)


def make_selw():
    s = np.zeros((P, 32), dtype=np.float16)
    for c in range(CH):
        for g in range(G):
            s[c * G + g, g] = 1.0
            s[c * G + g, 16 + g] = 1.0
    return s


def _apv(t_ap, pcnt, free_dims, free_off=0):
    pstride = t_ap.ap[0][0]
    return AP(t_ap.tensor, t_ap.offset + free_off,
              [[pstride, pcnt]] + [list(d) for d in free_dims])


def _dram_ap(handle, offset, dims):
    a = handle[:]
    return AP(a.tensor, a.offset + offset, [list(d) for d in dims])


def emit(nc: bass.Bass, cfg: Cfg):
    HS, WB, WBH, Hp = cfg.HS, cfg.WB, cfg.WBH, cfg.Hp
    NQ, QF, SF = cfg.NQ, cfg.QF, cfg.SF
    NSLAB, NB, NC, NS = cfg.NSLAB, cfg.NB, cfg.NC, cfg.NS
    Act = mybir.ActivationFunctionType

    dp = nc.declare_dram_parameter
    q_d = dp("q_d", [2, CH, NQ], F16, isOutput=False)    # query planes per term
    r_d = dp("r_d", [2, CH, NQ], F16, isOutput=False)    # ref planes per term
    selw_d = dp("selw_d", [P, 32], F16, isOutput=False)
    out_d = dp("out_d", [128, 1], F32, isOutput=True)

    with ExitStack() as ex:
        E = ex.enter_context
        q_s = [[E(nc.sbuf_tensor(f"q{t}{p}", [P, QF + 2 * R], F16))
                for p in range(2)] for t in range(2)]
        r_s = [[E(nc.sbuf_tensor(f"r{t}{p}", [P, SF], F16))
                for p in range(2)] for t in range(2)]
        sq_s = [E(nc.sbuf_tensor(f"sq{i}", [P, SF], F16)) for i in range(NSQ)]
        acc_s = E(nc.sbuf_tensor("acc", [128, cfg.NEXP], F32))
        res_s = E(nc.sbuf_tensor("res", [128, 1], F32))
        selw_s = E(nc.sbuf_tensor("selw", [P, 32], F16))
        ps_s = E(nc.psum_tensor("ps", [128, 4096], F32))

        sLC = E(nc.semaphore("sLC"))
        sL0 = E(nc.semaphore("sL0"))
        sL1 = E(nc.semaphore("sL1"))
        sG = E(nc.semaphore("sG"))
        sV = E(nc.semaphore("sV"))    # DVE subs (+ final reduce)
        sQD = E(nc.semaphore("sQD"))  # DVE squares
        sQA = E(nc.semaphore("sQA"))  # ACT squares
        sQP = E(nc.semaphore("sQP"))  # Pool squares
        sP = E(nc.semaphore("sP"))    # PE bank (unit) completions
        sE = E(nc.semaphore("sE"))    # exp ops done
        blk = E(nc.Block())

        sqsem = {'D': sQD, 'A': sQA, 'P': sQP}

        def sub_aps(s, j):
            t, dy, dx = cfg.slots[j]
            ph = s % 2
            gj = s * NS + j
            sq = sq_s[gj % NSQ].ap()
            r = _apv(r_s[t][ph].ap(), P, [[1, SF]])
            qoff = (2 + dy) * WBH + 2 + dx
            q = _apv(q_s[t][ph].ap(), P, [[1, SF]], qoff)
            return sq, r, q

        # ---------------- SP: DMA ----------------
        @blk.sync
        def _(sp):
            sp.dma_start(selw_s[:], selw_d[:]).then_inc(sLC, 16)
            for s in range(NSLAB):
                ph = s % 2
                r0 = s * HS
                sLs = sL0 if s % 2 == 0 else sL1
                if s >= 2:
                    sp.wait_ge(sV, NS * (s - 1))
                for t in range(2):
                    sp.dma_start(
                        _apv(q_s[t][ph].ap(), P, [[1, QF]]),
                        _dram_ap(q_d, t * CH * NQ + r0 * WBH,
                                 [[NQ, CH], [Hp * WBH, G], [1, QF]])
                    ).then_inc(sLs, 16)
                    sp.dma_start(
                        r_s[t][ph].ap(),
                        _dram_ap(r_d, t * CH * NQ + (r0 + 2) * WBH,
                                 [[NQ, CH], [Hp * WBH, G], [1, SF]])
                    ).then_inc(sLs, 16)
            sp.wait_ge(sV, cfg.NSLOT + 1)
            sp.dma_start(out_d[:], res_s.ap()).then_inc(sLC, 16)

        # ---------------- DVE: subs + its squares + final reduce ----------
        @blk.vector
        def _(ve):
            ve.wait_ge(sG, 1)
            for s in range(NSLAB):
                sLs = sL0 if s % 2 == 0 else sL1
                ve.wait_ge(sLs, 64 * (s // 2 + 1))
                for j in range(NS):
                    gj = s * NS + j
                    if gj >= NSQ:
                        ve.wait_ge(sP, NC * (gj - NSQ + 1))
                    sq, r, q = sub_aps(s, j)
                    nc.vector.tensor_tensor(
                        sq, r, q, AluOpType.subtract).then_inc(sV, 1)
                    if cfg.assign[j] == 'D':
                        ve.wait_ge(sV, gj + 1)
                        nc.vector.tensor_mul(sq, sq, sq).then_inc(sQD, 1)
            ve.wait_ge(sE, cfg.NEXP)
            nc.vector.tensor_reduce(
                res_s.ap(), acc_s.ap(), axis=mybir.AxisListType.X,
                op=AluOpType.add).then_inc(sV, 1)

        # ---------------- PE: selector matmuls ----------------
        @blk.tensor
        def _(pe):
            pe.wait_ge(sLC, 16)
            for gb in range(cfg.GB):
                s, b = divmod(gb, NB)
                bslots = cfg.batches[b]
                for jj in range(len(bslots)):
                    j = b * SBATCH + jj
                    gj = s * NS + j
                    e = cfg.assign[j]
                    pe.wait_ge(sqsem[e], cfg.cum[e][gj])
                    tile = sq_s[gj % NSQ].ap()
                    pstride = tile.ap[0][0]
                    for c, (ro, nr) in enumerate(cfg.chunks):
                        u = gb * NC + c
                        if jj == 0 and u >= 8:
                            pe.wait_ge(sE, cfg.expop_of_unit[u - 8] + 1)
                        k = u % 8
                        cols = nr * WB
                        rhs = AP(tile.tensor, tile.offset + ro * WBH + 2,
                                 [[pstride, P], [WBH, nr], [1, WB]])
                        nc.tensor.matmul(
                            ps_s[32 * jj:32 * jj + 32, 512 * k:512 * k + cols],
                            selw_s[:], rhs, start=True, stop=True,
                            skip_group_check=True, tile_position=(0, 32 * jj)
                        ).then_inc(sP, 1)

        # ---------------- ACT: its squares (2-batch lookahead) + exps ------
        @blk.scalar
        def _(ac):
            ac.wait_ge(sG, 1)
            ps_ap = ps_s.ap()
            ps_pstride = ps_ap.ap[0][0]
            nexp_done = 0
            for gbi in range(-2, cfg.GB):
                gb2 = gbi + 2
                if gb2 < cfg.GB:
                    s, b = divmod(gb2, NB)
                    for jj in range(len(cfg.batches[b])):
                        j = b * SBATCH + jj
                        if cfg.assign[j] != 'A':
                            continue
                        gj = s * NS + j
                        ac.wait_ge(sV, gj + 1)
                        sq = sq_s[gj % NSQ].ap()
                        nc.scalar.activation(
                            sq, sq, Act.Square).then_inc(sQA, 1)
                if gbi >= 0:
                    gb = gbi
                    s_, b_ = divmod(gb, NB)
                    last_gj = s_ * NS + b_ * SBATCH + len(cfg.batches[b_]) - 1
                    while (nexp_done < cfg.NEXP
                           and cfg.exp_ops[nexp_done][0] == gb):
                        (_, c0, c1, k0, cols, pb) = cfg.exp_ops[nexp_done]
                        ac.wait_ge(sP, NC * last_gj + c1 + 1)
                        nb_banks = c1 - c0 + 1
                        pa = AP(ps_ap.tensor, ps_ap.offset + 512 * k0,
                                [[ps_pstride, pb], [512, nb_banks], [1, cols]])
                        nc.scalar.activation(
                            pa, pa, Act.Exp, scale=EXP_SCALE,
                            accum_out=acc_s[:pb, nexp_done:nexp_done + 1]
                        ).then_inc(sE, 1)
                        nexp_done += 1

        # ---------------- Pool: acc memset + its squares ----------------
        @blk.gpsimd
        def _(gp):
            gp.memset(acc_s.ap(), 0.0)
            for t in range(2):
                for p in range(2):
                    gp.memset(q_s[t][p][:, QF:QF + 2 * R], 0.0)
            gp.drain()
            gp.sem_inc(sG, 1)
            for gj in range(cfg.NSLOT):
                if cfg.assign[gj % NS] != 'P':
                    continue
                gp.wait_ge(sV, gj + 1)
                sq = sq_s[gj % NSQ].ap()
                nc.gpsimd.tensor_tensor(
                    sq, sq, sq, AluOpType.mult).then_inc(sQP, 1)
    return nc


# ---------------- host side ----------------

def _block(plane, cfg, pad=0.0):
    """[H, W] -> flat blocked+haloed [G*Hp*WBH] fp16."""
    p = np.full((cfg.Hp, cfg.W + 2 * R), pad, dtype=np.float32)
    p[R:R + cfg.H, R:R + cfg.W] = plane
    out = np.empty((G, cfg.Hp, cfg.WBH), dtype=np.float16)
    for g in range(G):
        out[g] = p[:, g * cfg.WB:g * cfg.WB + cfg.WBH]
    return out.reshape(-1)


def host_precompute(rgb, depth, depth_gt, depth_mask, depth_gt_mask,
                    xy1_grid, Ts, cfg, b):
    tb = b ^ 1
    xy1 = np.asarray(xy1_grid[b], np.float32)
    xy1_t = np.asarray(xy1_grid[tb], np.float32)
    dep = np.asarray(depth[b, 0], np.float32)
    dgt_b = np.asarray(depth_gt[b, 0], np.float32)
    dgt_t = np.asarray(depth_gt[tb, 0], np.float32)
    mp = np.asarray(depth_mask[b, 0], np.float32)
    mg_b = np.asarray(depth_gt_mask[b, 0], np.float32)
    mg_t = np.asarray(depth_gt_mask[tb, 0], np.float32)

    xyz_p = xy1 * dep
    T21 = (np.linalg.inv(np.asarray(Ts[tb], np.float64)) @
           np.asarray(Ts[b], np.float64)).astype(np.float32)
    Rm, tv = T21[:3, :3], T21[:3, 3]
    txyz = np.einsum('ij,jhw->ihw', Rm, xyz_p).astype(np.float32) \
        + tv[:, None, None].astype(np.float32)
    pos = (txyz[2] > 0).astype(np.float32) * mp

    q = np.empty((2, CH, cfg.NQ), np.float16)
    r = np.empty((2, CH, cfg.NQ), np.float16)
    for c in range(3):
        q[0, c] = _block(xyz_p[c], cfg)
        q[1, c] = _block(txyz[c], cfg)
        r[0, c] = _block(xy1[c] * dgt_b, cfg)
        r[1, c] = _block(xy1_t[c] * dgt_t, cfg)
    for c in range(3):
        rgb_b = np.asarray(rgb[b, c], np.float32)
        q[0, 3 + c] = q[1, 3 + c] = _block(rgb_b, cfg)
        r[0, 3 + c] = q[0, 3 + c]
        r[1, 3 + c] = _block(np.asarray(rgb[tb, c], np.float32), cfg)
    # mask channels: query = -20*(1-m) (padding -> -20), ref = +20*(1-mg)
    q[0, 6] = (-M_C * (1.0 - _block(mp, cfg).astype(np.float32))).astype(np.float16)
    q[1, 6] = (-M_C * (1.0 - _block(pos, cfg).astype(np.float32))).astype(np.float16)
    r[0, 6] = (M_C * (1.0 - _block(mg_b, cfg).astype(np.float32))).astype(np.float16)
    r[1, 6] = (M_C * (1.0 - _block(mg_t, cfg).astype(np.float32))).astype(np.float16)
    return {"q_d": q, "r_d": r, "selw_d": make_selw()}


def make_in_maps(rgb, depth, depth_gt, depth_mask, depth_gt_mask, xy1_grid, Ts,
                 cfg, n_cores=8):
    return [host_precompute(rgb, depth, depth_gt, depth_mask, depth_gt_mask,
                            xy1_grid, Ts, cfg, b) for b in range(n_cores)]


_CACHED = {}


def _get_nc(cfg_key=(352, 1216, 32)):
    if cfg_key not in _CACHED:
        cfg = Cfg(*cfg_key)
        nc = bass.Bass()
        emit(nc, cfg)
        _CACHED[cfg_key] = (nc, cfg)
    return _CACHED[cfg_key]


def kernel(rgb, depth, depth_gt, depth_mask, depth_gt_mask, xy1_grid, Ts,
           **run_kwargs):
    from concourse.bass_utils import run_bass_kernel_spmd
    nc, cfg = _get_nc()
    maps = make_in_maps(rgb, depth, depth_gt, depth_mask, depth_gt_mask,
                        xy1_grid, Ts, cfg)
    res = run_bass_kernel_spmd(nc, maps, list(range(8)), **run_kwargs)
    total = np.float64(0.0)
    for r in res.results:
        total += np.float64(r["out_d"][:, 0].sum())
    total *= 0.5  # selector weights duplicate each block sum into 2 rows
    n_gt = max(np.asarray(depth_gt_mask, np.float64).sum(), 1.0)
    loss = -total / n_gt
    kernel.last_results = res
    return np.float32(loss)


# revision 12
# speedup vs baseline: 1.2536x; 1.2536x over previous
"""C3DLoss kernel for Trainium2 — 8-core batch-parallel, raw-Bass implementation.

Per core = one batch frame b (pairing partner tb = b^1):
    partial = sum over terms t in {same, cross}, shifts d in [-2,2]^2, pixels p:
        exp(-50 * |feat_r(p) - feat_q(p+d)|^2)
    feat = (x, y, z', r, g, b) with the masks FOLDED INTO z:
        z'_ref   = z_ref   - 120*(1 - mask_ref)
        z'_query = z_query -  40*(1 - mask_query)
    With z in (0, 10.2], any off-mask combination leaves |dz| >= ~30, so
    d2 >= ~900 and exp(-50*d2) = 0 exactly like the reference's mask
    product; with both masks on, z' = z exactly.
    loss = -(sum of partials - garbage_const) / max(sum(depth_gt_mask), 1).

Device mapping (v3):
  - Host pre-blocks every plane into G=19 W-blocks of width 64 with +-2 halo
    in both dims, fp16. 6 channels x 19 blocks = 114 partitions in ONE tile:
    each shift slot costs one DVE subtract (fp16, 2x mode) + one square.
  - Squares statically split across DVE / ACT(Square) / Pool(GPSIMD) to
    balance engine load.
  - PE reduces channels with [114, 32] selector weights (cols 19..31 zero);
    4 slots pack a 128-partition PSUM bank set; chunks are 8 rows x 512
    cols = exactly one PSUM bank, 4 banks per 4-slot batch, so batches
    alternate cleanly between banks {0-3} and {4-7}.
  - ACT computes exp(-50*d2) in place on PSUM, one op per batch over all 4
    banks ([*, 2048] contiguous), accum into acc[:, batch]. Zero-weight
    output partitions contribute exp(0)=1 each; that deterministic constant
    is subtracted on the host.
"""

import sys

for _p in ("/opt/trn_rl_repo", "/opt/pypackages"):
    if _p not in sys.path:
        sys.path.insert(0, _p)

from contextlib import ExitStack

import numpy as np

import concourse.bass as bass
import concourse.mybir as mybir
from concourse.ap import AP
from concourse.alu_op_type import AluOpType

F16 = mybir.dt.float16
F32 = mybir.dt.float32

R = 2
G = 19            # W-blocks
CH = 6            # x, y, z', r, g, b
P = CH * G        # 114 partitions per feature tile
SBATCH = 4        # slots per PSUM bank set
NSQ = 16          # rotating diff/square tiles
MREF_C = 120.0    # ref-side mask fold magnitude
MQ_C = 40.0       # query-side mask fold magnitude
EXP_SCALE = -50.0

# per-slab square-engine assignment counts (DVE, ACT, Pool)
N_SQ_DVE = 10
N_SQ_ACT = 23
N_SQ_POOL = 17


class Cfg:
    def __init__(self, H=352, W=1216, HS=32):
        assert W % G == 0 and H % HS == 0
        self.H, self.W, self.HS = H, W, HS
        self.WB = W // G                       # 64
        self.WBH = self.WB + 2 * R             # 68
        self.Hp = H + 2 * R                    # 356
        self.NSLAB = H // HS                   # 11
        self.NQ = G * self.Hp * self.WBH       # haloed plane elems
        self.QF = (HS + 2 * R) * self.WBH      # query tile free size 2448
        self.SF = HS * self.WBH                # slab tile free size 2176
        cr = 512 // self.WB                    # 8 rows -> 512 cols
        assert HS % cr == 0
        self.chunks = [(o, cr) for o in range(0, HS, cr)]
        self.NC = len(self.chunks)             # 4
        assert self.NC * 2 == 8                # two alternating bank sets
        self.slots = [(t, dy, dx) for t in (0, 1)
                      for dy in range(-R, R + 1) for dx in range(-R, R + 1)]
        self.NS = len(self.slots)              # 50
        self.batches = [self.slots[i:i + SBATCH]
                        for i in range(0, self.NS, SBATCH)]
        self.NB = len(self.batches)            # 13
        self.GB = self.NSLAB * self.NB         # 143 global batches
        self.NEXP = self.GB                    # one exp op per batch
        self.NSLOT = self.NSLAB * self.NS

        # square-engine assignment per slab slot (largest-remainder interleave)
        want = {'D': N_SQ_DVE, 'A': N_SQ_ACT, 'P': N_SQ_POOL}
        assert sum(want.values()) == self.NS
        emitted = {k: 0 for k in want}
        self.assign = []
        for j in range(self.NS):
            k = max(want, key=lambda e: want[e] * (j + 1) / self.NS - emitted[e])
            emitted[k] += 1
            self.assign.append(k)
        self.cum = {'D': [], 'A': [], 'P': []}
        tot = {'D': 0, 'A': 0, 'P': 0}
        for gj in range(self.NSLOT):
            e = self.assign[gj % self.NS]
            tot[e] += 1
            for k in self.cum:
                self.cum[k].append(tot[k])

    def garbage_const(self):
        """exp(0)=1 contributions from the 13 zero-weight output partitions
        of each 32-partition slot group."""
        tot = 0
        for gb in range(self.GB):
            nslots = len(self.batches[gb % self.NB])
            tot += nslots * (32 - G) * self.HS * self.WB
        return float(tot)


def make_selw():
    s = np.zeros((P, 32), dtype=np.float16)
    for c in range(CH):
        for g in range(G):
            s[c * G + g, g] = 1.0
    return s


def _apv(t_ap, pcnt, free_dims, free_off=0):
    pstride = t_ap.ap[0][0]
    return AP(t_ap.tensor, t_ap.offset + free_off,
              [[pstride, pcnt]] + [list(d) for d in free_dims])


def _dram_ap(handle, offset, dims):
    a = handle[:]
    return AP(a.tensor, a.offset + offset, [list(d) for d in dims])


def emit(nc: bass.Bass, cfg: Cfg):
    HS, WB, WBH, Hp = cfg.HS, cfg.WB, cfg.WBH, cfg.Hp
    NQ, QF, SF = cfg.NQ, cfg.QF, cfg.SF
    NSLAB, NB, NC, NS = cfg.NSLAB, cfg.NB, cfg.NC, cfg.NS
    Act = mybir.ActivationFunctionType

    dp = nc.declare_dram_parameter
    q_d = dp("q_d", [2, CH, NQ], F16, isOutput=False)    # query planes per term
    r_d = dp("r_d", [2, CH, NQ], F16, isOutput=False)    # ref planes per term
    selw_d = dp("selw_d", [P, 32], F16, isOutput=False)
    out_d = dp("out_d", [128, 1], F32, isOutput=True)

    with ExitStack() as ex:
        E = ex.enter_context
        q_s = [[E(nc.sbuf_tensor(f"q{t}{p}", [P, QF + 2 * R], F16))
                for p in range(2)] for t in range(2)]
        r_s = [[E(nc.sbuf_tensor(f"r{t}{p}", [P, SF], F16))
                for p in range(2)] for t in range(2)]
        sq_s = [E(nc.sbuf_tensor(f"sq{i}", [P, SF], F16)) for i in range(NSQ)]
        acc_s = E(nc.sbuf_tensor("acc", [128, cfg.NEXP], F32))
        res_s = E(nc.sbuf_tensor("res", [128, 1], F32))
        selw_s = E(nc.sbuf_tensor("selw", [P, 32], F16))
        ps_s = E(nc.psum_tensor("ps", [128, 4096], F32))

        sLC = E(nc.semaphore("sLC"))
        sL0 = E(nc.semaphore("sL0"))
        sL1 = E(nc.semaphore("sL1"))
        sG = E(nc.semaphore("sG"))
        sV = E(nc.semaphore("sV"))    # DVE subs (+ final reduce)
        sQD = E(nc.semaphore("sQD"))  # DVE squares
        sQA = E(nc.semaphore("sQA"))  # ACT squares
        sQP = E(nc.semaphore("sQP"))  # Pool squares
        sP = E(nc.semaphore("sP"))    # PE matmul completions (4 per slot)
        sE = E(nc.semaphore("sE"))    # exp ops done (1 per batch)
        blk = E(nc.Block())

        sqsem = {'D': sQD, 'A': sQA, 'P': sQP}

        def sub_aps(s, j):
            t, dy, dx = cfg.slots[j]
            ph = s % 2
            gj = s * NS + j
            sq = sq_s[gj % NSQ].ap()
            r = _apv(r_s[t][ph].ap(), P, [[1, SF]])
            qoff = (2 + dy) * WBH + 2 + dx
            q = _apv(q_s[t][ph].ap(), P, [[1, SF]], qoff)
            return sq, r, q

        # ---------------- SP: DMA ----------------
        @blk.sync
        def _(sp):
            sp.dma_start(selw_s[:], selw_d[:]).then_inc(sLC, 16)
            for s in range(NSLAB):
                ph = s % 2
                r0 = s * HS
                sLs = sL0 if s % 2 == 0 else sL1
                if s >= 2:
                    sp.wait_ge(sV, NS * (s - 1))
                for t in range(2):
                    sp.dma_start(
                        _apv(q_s[t][ph].ap(), P, [[1, QF]]),
                        _dram_ap(q_d, t * CH * NQ + r0 * WBH,
                                 [[NQ, CH], [Hp * WBH, G], [1, QF]])
                    ).then_inc(sLs, 16)
                    sp.dma_start(
                        r_s[t][ph].ap(),
                        _dram_ap(r_d, t * CH * NQ + (r0 + 2) * WBH,
                                 [[NQ, CH], [Hp * WBH, G], [1, SF]])
                    ).then_inc(sLs, 16)
            sp.wait_ge(sV, cfg.NSLOT + 1)
            sp.dma_start(out_d[:], res_s.ap()).then_inc(sLC, 16)

        # ---------------- DVE: subs + its squares + final reduce ----------
        @blk.vector
        def _(ve):
            ve.wait_ge(sG, 1)
            for s in range(NSLAB):
                sLs = sL0 if s % 2 == 0 else sL1
                ve.wait_ge(sLs, 64 * (s // 2 + 1))
                for j in range(NS):
                    gj = s * NS + j
                    if gj >= NSQ:
                        ve.wait_ge(sP, NC * (gj - NSQ + 1))
                    sq, r, q = sub_aps(s, j)
                    nc.vector.tensor_tensor(
                        sq, r, q, AluOpType.subtract).then_inc(sV, 1)
                    if cfg.assign[j] == 'D':
                        ve.wait_ge(sV, gj + 1)
                        nc.vector.tensor_mul(sq, sq, sq).then_inc(sQD, 1)
            ve.wait_ge(sE, cfg.NEXP)
            nc.vector.tensor_reduce(
                res_s.ap(), acc_s.ap(), axis=mybir.AxisListType.X,
                op=AluOpType.add).then_inc(sV, 1)

        # ---------------- PE: selector matmuls ----------------
        @blk.tensor
        def _(pe):
            pe.wait_ge(sLC, 16)
            for gb in range(cfg.GB):
                s, b = divmod(gb, NB)
                bslots = cfg.batches[b]
                if gb >= 2:
                    pe.wait_ge(sE, gb - 1)   # bank set free (exp of gb-2)
                k0 = (gb * NC) % 8
                for jj in range(len(bslots)):
                    j = b * SBATCH + jj
                    gj = s * NS + j
                    e = cfg.assign[j]
                    pe.wait_ge(sqsem[e], cfg.cum[e][gj])
                    tile = sq_s[gj % NSQ].ap()
                    pstride = tile.ap[0][0]
                    for c, (ro, nr) in enumerate(cfg.chunks):
                        k = k0 + c
                        rhs = AP(tile.tensor, tile.offset + ro * WBH + 2,
                                 [[pstride, P], [WBH, nr], [1, WB]])
                        nc.tensor.matmul(
                            ps_s[32 * jj:32 * jj + 32, 512 * k:512 * (k + 1)],
                            selw_s[:], rhs, start=True, stop=True,
                            skip_group_check=True, tile_position=(0, 32 * jj)
                        ).then_inc(sP, 1)

        # ---------------- ACT: its squares (2-batch lookahead) + exps ------
        @blk.scalar
        def _(ac):
            ac.wait_ge(sG, 1)
            ps_ap = ps_s.ap()
            ps_pstride = ps_ap.ap[0][0]
            for gbi in range(-2, cfg.GB):
                gb2 = gbi + 2
                if gb2 < cfg.GB:
                    s, b = divmod(gb2, NB)
                    for jj in range(len(cfg.batches[b])):
                        j = b * SBATCH + jj
                        if cfg.assign[j] != 'A':
                            continue
                        gj = s * NS + j
                        ac.wait_ge(sV, gj + 1)
                        sq = sq_s[gj % NSQ].ap()
                        nc.scalar.activation(
                            sq, sq, Act.Square).then_inc(sQA, 1)
                if gbi >= 0:
                    gb = gbi
                    s_, b_ = divmod(gb, NB)
                    nslots = len(cfg.batches[b_])
                    last_gj = s_ * NS + b_ * SBATCH + nslots - 1
                    ac.wait_ge(sP, NC * (last_gj + 1))
                    k0 = (gb * NC) % 8
                    pb = 32 * nslots
                    pa = AP(ps_ap.tensor, ps_ap.offset + 512 * k0,
                            [[ps_pstride, pb], [1, 512 * NC]])
                    nc.scalar.activation(
                        pa, pa, Act.Exp, scale=EXP_SCALE,
                        accum_out=acc_s[:pb, gb:gb + 1]).then_inc(sE, 1)

        # ---------------- Pool: init memsets + its squares ----------------
        @blk.gpsimd
        def _(gp):
            gp.memset(acc_s.ap(), 0.0)
            for t in range(2):
                for p in range(2):
                    gp.memset(q_s[t][p][:, QF:QF + 2 * R], 0.0)
            gp.drain()
            gp.sem_inc(sG, 1)
            for gj in range(cfg.NSLOT):
                if cfg.assign[gj % NS] != 'P':
                    continue
                gp.wait_ge(sV, gj + 1)
                sq = sq_s[gj % NSQ].ap()
                nc.gpsimd.tensor_tensor(
                    sq, sq, sq, AluOpType.mult).then_inc(sQP, 1)
    return nc


# ---------------- host side ----------------

def _block(plane, cfg, pad=0.0):
    """[H, W] -> flat blocked+haloed [G*Hp*WBH] fp16."""
    p = np.full((cfg.Hp, cfg.W + 2 * R), pad, dtype=np.float32)
    p[R:R + cfg.H, R:R + cfg.W] = plane
    out = np.empty((G, cfg.Hp, cfg.WBH), dtype=np.float16)
    for g in range(G):
        out[g] = p[:, g * cfg.WB:g * cfg.WB + cfg.WBH]
    return out.reshape(-1)


def host_precompute(rgb, depth, depth_gt, depth_mask, depth_gt_mask,
                    xy1_grid, Ts, cfg, b):
    tb = b ^ 1
    xy1 = np.asarray(xy1_grid[b], np.float32)
    xy1_t = np.asarray(xy1_grid[tb], np.float32)
    dep = np.asarray(depth[b, 0], np.float32)
    dgt_b = np.asarray(depth_gt[b, 0], np.float32)
    dgt_t = np.asarray(depth_gt[tb, 0], np.float32)
    mp = np.asarray(depth_mask[b, 0], np.float32)
    mg_b = np.asarray(depth_gt_mask[b, 0], np.float32)
    mg_t = np.asarray(depth_gt_mask[tb, 0], np.float32)

    xyz_p = xy1 * dep
    T21 = (np.linalg.inv(np.asarray(Ts[tb], np.float64)) @
           np.asarray(Ts[b], np.float64)).astype(np.float32)
    Rm, tv = T21[:3, :3], T21[:3, 3]
    txyz = np.einsum('ij,jhw->ihw', Rm, xyz_p).astype(np.float32) \
        + tv[:, None, None].astype(np.float32)
    pos = (txyz[2] > 0).astype(np.float32) * mp

    q = np.empty((2, CH, cfg.NQ), np.float16)
    r = np.empty((2, CH, cfg.NQ), np.float16)
    for c in range(2):
        q[0, c] = _block(xyz_p[c], cfg)
        q[1, c] = _block(txyz[c], cfg)
        r[0, c] = _block(xy1[c] * dgt_b, cfg)
        r[1, c] = _block(xy1_t[c] * dgt_t, cfg)
    # z' channels: mask folded in; query padding = mask-off (-MQ_C)
    q[0, 2] = _block(xyz_p[2] - MQ_C * (1.0 - mp), cfg, pad=-MQ_C)
    q[1, 2] = _block(txyz[2] - MQ_C * (1.0 - pos), cfg, pad=-MQ_C)
    r[0, 2] = _block(xy1[2] * dgt_b - MREF_C * (1.0 - mg_b), cfg)
    r[1, 2] = _block(xy1_t[2] * dgt_t - MREF_C * (1.0 - mg_t), cfg)
    for c in range(3):
        rgb_b = np.asarray(rgb[b, c], np.float32)
        q[0, 3 + c] = q[1, 3 + c] = _block(rgb_b, cfg)
        r[0, 3 + c] = q[0, 3 + c]
        r[1, 3 + c] = _block(np.asarray(rgb[tb, c], np.float32), cfg)
    return {"q_d": q, "r_d": r, "selw_d": make_selw()}


def make_in_maps(rgb, depth, depth_gt, depth_mask, depth_gt_mask, xy1_grid, Ts,
                 cfg, n_cores=8):
    return [host_precompute(rgb, depth, depth_gt, depth_mask, depth_gt_mask,
                            xy1_grid, Ts, cfg, b) for b in range(n_cores)]


_CACHED = {}


def _get_nc(cfg_key=(352, 1216, 32)):
    if cfg_key not in _CACHED:
        cfg = Cfg(*cfg_key)
        nc = bass.Bass()
        emit(nc, cfg)
        _CACHED[cfg_key] = (nc, cfg)
    return _CACHED[cfg_key]


def kernel(rgb, depth, depth_gt, depth_mask, depth_gt_mask, xy1_grid, Ts,
           **run_kwargs):
    from concourse.bass_utils import run_bass_kernel_spmd
    nc, cfg = _get_nc()
    maps = make_in_maps(rgb, depth, depth_gt, depth_mask, depth_gt_mask,
                        xy1_grid, Ts, cfg)
    res = run_bass_kernel_spmd(nc, maps, list(range(8)), **run_kwargs)
    garbage = cfg.garbage_const()
    total = np.float64(0.0)
    for r in res.results:
        total += np.float64(r["out_d"][:, 0].sum()) - garbage
    n_gt = max(np.asarray(depth_gt_mask, np.float64).sum(), 1.0)
    loss = -total / n_gt
    kernel.last_results = res
    return np.float32(loss)
